# revision 1
# baseline (speedup 1.0000x reference)
"""2-layer GCN encoder (PyG GCNConv semantics) on 8 Trainium2 NeuronCores.

  out_l = relu(dinv * (A_hat @ u_l) + b_l),  u_l = (dinv * in_l) @ W_l
  A_hat includes self loops; dinv = deg^-1/2 (deg incl. self loop).

Layout: nodes are relabelled by a degree-balancing permutation, padded to
NP = 8*SHARD, and partitioned into 784 dst tiles of 128 (98 tiles per core).

Layer 1: the host pre-gathers x*dinv rows into edge order (grouped by dst
tile, padded per tile); the device streams them contiguously, scatter-adds
S^T per tile via one-hot matmuls (lhsT=M, rhs=P), then applies W1, the
relu/dinv epilogue, and W2 to produce the layer-2 source u2 directly.

Layer 2: u2 shards are AllGathered (bf16), then each core row-gathers
u2_full[src] for its dst tiles with batched dma_gather (4 SWDGE queues,
int16 indices relative to 4 source blocks), scatter-adds via one-hot
matmuls (lhsT=P, rhs=M), and writes relu(dinv*S) output tiles.
"""

import time
from contextlib import ExitStack
from dataclasses import dataclass, replace

import numpy as np
import ml_dtypes

import concourse.bass as bass
import concourse.bacc as bacc
import concourse.mybir as mybir
import concourse.tile as tile
from concourse.bass_utils import run_bass_kernel_spmd

BF16 = ml_dtypes.bfloat16
P = 128


@dataclass(frozen=True)
class Cfg:
    n_cores: int = 8
    d: int = 128
    n_real: int = 100000
    shard: int = 12544       # nodes per core, multiple of 128
    b: int = 7               # dst tiles per batch
    grp: int = 4             # layer-2 source blocks (int16 index range)
    cap: int = 640           # layer-2 edge slots per (tile, group), mult of 128
    cap1: int = 2304         # layer-1 edge slots per tile, multiple of 128

    @property
    def np_(self):
        return self.n_cores * self.shard

    @property
    def tiles(self):
        return self.shard // P

    @property
    def nb(self):
        return self.tiles // self.b

    @property
    def blk(self):
        return self.np_ // self.grp

    @property
    def ch(self):
        return self.cap // P

    @property
    def chb(self):
        return self.b * self.ch

    @property
    def kb(self):
        return self.grp * self.chb       # L2 chunks per batch

    @property
    def ncall(self):
        return self.b * self.cap

    @property
    def wcols(self):
        return self.ncall // 16

    @property
    def ch1(self):
        return self.cap1 // P

    @property
    def kb1(self):
        return self.b * self.ch1         # L1 chunks per batch


FULL_CFG = Cfg()

LAST_INFO: dict = {}


def install_ntff_hook():
    """Provide antenv.axon_hooks (absent on this image) so that
    run_bass_kernel_spmd(trace=True) can capture NTFF profiles."""
    import sys
    import types

    if "antenv.axon_hooks" in sys.modules:
        return
    mod = types.ModuleType("antenv.axon_hooks")
    holder = [None]
    mod.set_axon_ntff_profile_hook = lambda h: holder.__setitem__(0, h)
    mod.get_axon_ntff_profile_hook = lambda: holder[0]
    sys.modules["antenv.axon_hooks"] = mod
    try:
        import antenv

        antenv.axon_hooks = mod
    except ImportError:
        pass
    try:
        from trn_agent_boot.trn_boot import _ntff_profile_via_ctypes

        hook = _ntff_profile_via_ctypes("/opt/axon/libaxon_pjrt.so")
        if hook is not None:
            mod.set_axon_ntff_profile_hook(hook)
    except Exception as e:  # profiling optional
        print(f"NTFF hook install failed: {e}")


def _check_cfg(cfg: Cfg):
    assert cfg.shard % P == 0 and cfg.tiles % cfg.b == 0
    assert cfg.cap % P == 0 and cfg.cap1 % P == 0
    assert cfg.np_ % cfg.grp == 0
    assert cfg.blk <= 32768, "relative gather indices must fit int16"
    assert cfg.ncall % 16 == 0


def preprocess(x, edge_index, W1, b1, W2, b2, cfg: Cfg):
    N, D = cfg.n_real, cfg.d
    NP = cfg.np_
    assert x.shape == (N, D)

    src0 = np.asarray(edge_index[0]).astype(np.int64)
    dst0 = np.asarray(edge_index[1]).astype(np.int64)
    loops = np.arange(N, dtype=np.int64)
    src0 = np.concatenate([src0, loops])
    dst0 = np.concatenate([dst0, loops])

    deg0 = np.bincount(dst0, minlength=N).astype(np.float32)

    # degree-balancing relabelling: snake-deal nodes (sorted by in-degree)
    # across the tiles so every tile gets ~equal total degree.
    ntiles = NP // P
    order_by_deg = np.argsort(-deg0, kind="stable")          # real nodes
    dealt = np.full(P * ntiles, -1, np.int64)
    dealt[:N] = order_by_deg
    dealt = dealt.reshape(P, ntiles)
    dealt[1::2] = dealt[1::2, ::-1]                          # snake rounds
    # node dealt[r, t] -> new id t*128 + r
    new_of = np.full(N, -1, np.int64)
    rr, tt = np.nonzero(dealt >= 0)
    new_ids = tt * P + rr
    new_of[dealt[rr, tt]] = new_ids
    orig_of = np.full(NP, -1, np.int64)
    orig_of[new_ids] = dealt[rr, tt]

    src = new_of[src0]
    dst = new_of[dst0]

    deg = np.zeros(NP, np.float32)
    deg[new_ids] = deg0[dealt[rr, tt]]
    dinv = np.zeros(NP, np.float32)
    nz = deg > 0
    dinv[nz] = 1.0 / np.sqrt(deg[nz])

    # scaled features in new numbering (pad rows zero)
    xs = np.zeros((NP, D), np.float32)
    dinv0 = np.zeros(N, np.float32)
    dinv0[deg0 > 0] = 1.0 / np.sqrt(deg0[deg0 > 0])
    xs[new_of] = np.asarray(x, np.float32) * dinv0[:, None]
    xs_bf = xs.astype(BF16)

    nc_, nb, b, grp = cfg.n_cores, cfg.nb, cfg.b, cfg.grp

    # ---- layer 1: host-pregathered message stream, grouped by dst tile ----
    t_of = dst >> 7
    order1 = np.argsort(t_of, kind="stable")
    d1 = dst[order1]
    s1 = src[order1]
    k1 = t_of[order1]
    cnt1 = np.bincount(k1, minlength=ntiles)
    need1 = int(np.ceil(cnt1.max() / P)) * P
    if need1 > cfg.cap1:
        cfg = replace(cfg, cap1=need1)
    starts1 = np.zeros(ntiles + 1, np.int64)
    starts1[1:] = np.cumsum(cnt1)
    pos1 = np.arange(len(s1)) - starts1[k1]
    dest1 = k1 * cfg.cap1 + pos1

    msg = np.zeros((ntiles * cfg.cap1, D), BF16)
    msg[dest1] = xs_bf[s1]
    dloc1_flat = np.full(ntiles * cfg.cap1, -1.0, BF16)
    dloc1_flat[dest1] = (d1 & 127).astype(BF16)

    ch1 = cfg.cap1 // P
    kb1 = b * ch1
    # [c, nb, tb, ch1, p, f] -> [c, nb, p, tb, ch1, f]
    m1_in = np.ascontiguousarray(
        msg.reshape(nc_, nb, b, ch1, P, D).transpose(0, 1, 4, 2, 3, 5)
    ).reshape(nc_, nb * P, kb1 * D)
    dloc1_in = np.ascontiguousarray(
        dloc1_flat.reshape(nc_, nb, b, ch1, P)
        .transpose(0, 4, 1, 2, 3)
        .reshape(nc_, P, nb * kb1)
    )

    # ---- layer 2: gather structures (on new ids) ----
    key = (dst >> 7) * grp + src // cfg.blk
    order = np.argsort(key, kind="stable")
    ks = key[order]
    ss = src[order]
    ds = dst[order]
    nkeys = ntiles * grp
    counts = np.bincount(ks, minlength=nkeys).astype(np.int64)
    need = int(np.ceil(counts.max() / P)) * P
    if need > cfg.cap:
        cfg = replace(cfg, cap=need)
    _check_cfg(cfg)
    cap = cfg.cap
    ch = cfg.ch

    starts = np.zeros(nkeys + 1, np.int64)
    starts[1:] = np.cumsum(counts)
    pos = np.arange(len(ss)) - starts[ks]
    dest = ks * cap + pos

    epad = nkeys * cap
    idx_flat = np.zeros(epad, np.int16)
    idx_flat[dest] = (ss - (ks % grp) * cfg.blk).astype(np.int16)
    dloc_flat = np.full(epad, -1.0, dtype=BF16)
    dloc_flat[dest] = (ds & 127).astype(BF16)

    idx5 = idx_flat.reshape(nc_, nb, b, grp, cap)
    idx_call = idx5.transpose(0, 1, 3, 2, 4).reshape(nc_, nb, grp, b * cap)
    wr = idx_call.reshape(nc_, nb, grp, cfg.wcols, 16).transpose(0, 1, 2, 4, 3)
    wr = np.tile(wr, (1, 1, 1, 8, 1))
    idxs_in = np.ascontiguousarray(
        wr.transpose(0, 3, 1, 2, 4).reshape(nc_, P, nb * grp * cfg.wcols)
    )

    dl6 = dloc_flat.reshape(nc_, nb, b, grp, ch, P)
    dl = dl6.transpose(0, 1, 5, 3, 2, 4).reshape(nc_, nb, P, grp * b * ch)
    dloc_in = np.ascontiguousarray(
        dl.transpose(0, 2, 1, 3).reshape(nc_, P, nb * cfg.kb)
    )

    dinv2 = dinv * dinv
    sc1_in = np.ascontiguousarray(
        dinv2.reshape(nc_, cfg.tiles, P).transpose(0, 2, 1)
    ).astype(np.float32)
    sc2_in = np.ascontiguousarray(
        dinv.reshape(nc_, cfg.tiles, P).transpose(0, 2, 1)
    ).astype(np.float32)

    iota_in = np.tile(np.arange(P, dtype=BF16)[None, :], (P, 1))
    ident_in = np.eye(P, dtype=np.float32)
    w1_in = np.asarray(W1, np.float32).astype(BF16)
    w2_in = np.asarray(W2, np.float32).astype(BF16)

    b1 = np.asarray(b1, np.float32)
    b2 = np.asarray(b2, np.float32)
    with_bias = bool(np.any(b1 != 0) or np.any(b2 != 0))
    sqrtdeg = np.sqrt(deg)

    in_maps = []
    for c in range(nc_):
        m = {
            "m1": m1_in[c],
            "dloc1": dloc1_in[c],
            "w1": w1_in,
            "w2": w2_in,
            "iota": iota_in,
            "ident": ident_in,
            "idxs": idxs_in[c],
            "dloc": dloc_in[c],
            "sc1": sc1_in[c],
            "sc2": sc2_in[c],
        }
        if with_bias:
            sh = slice(c * cfg.shard, (c + 1) * cfg.shard)
            m["bpre1"] = np.ascontiguousarray(np.outer(sqrtdeg[sh], b1)).astype(
                np.float32
            )
            m["bpre2"] = np.ascontiguousarray(np.outer(sqrtdeg[sh], b2)).astype(
                np.float32
            )
        in_maps.append(m)
    return in_maps, with_bias, cfg, orig_of


def build_program(cfg: Cfg, with_bias: bool):
    _check_cfg(cfg)
    D = cfg.d
    dt = mybir.dt
    Relu = mybir.ActivationFunctionType.Relu

    nc = bacc.Bacc(
        "TRN2",
        target_bir_lowering=False,
        debug=False,
        num_devices=cfg.n_cores,
        num_swdge_queues=4,
    )

    m1 = nc.dram_tensor(
        "m1", [cfg.nb * P, cfg.kb1 * D], dt.bfloat16, kind="ExternalInput"
    ).ap()
    dloc1 = nc.dram_tensor(
        "dloc1", [P, cfg.nb * cfg.kb1], dt.bfloat16, kind="ExternalInput"
    ).ap()
    w1 = nc.dram_tensor("w1", [D, D], dt.bfloat16, kind="ExternalInput").ap()
    w2 = nc.dram_tensor("w2", [D, D], dt.bfloat16, kind="ExternalInput").ap()
    iota = nc.dram_tensor("iota", [P, P], dt.bfloat16, kind="ExternalInput").ap()
    ident = nc.dram_tensor("ident", [P, P], dt.float32, kind="ExternalInput").ap()
    idxs = nc.dram_tensor(
        "idxs", [P, cfg.nb * cfg.grp * cfg.wcols], dt.int16, kind="ExternalInput"
    ).ap()
    dloc = nc.dram_tensor(
        "dloc", [P, cfg.nb * cfg.kb], dt.bfloat16, kind="ExternalInput"
    ).ap()
    sc1 = nc.dram_tensor("sc1", [P, cfg.tiles], dt.float32, kind="ExternalInput").ap()
    sc2 = nc.dram_tensor("sc2", [P, cfg.tiles], dt.float32, kind="ExternalInput").ap()
    if with_bias:
        bpre1 = nc.dram_tensor(
            "bpre1", [cfg.shard, D], dt.float32, kind="ExternalInput"
        ).ap()
        bpre2 = nc.dram_tensor(
            "bpre2", [cfg.shard, D], dt.float32, kind="ExternalInput"
        ).ap()
    out = nc.dram_tensor("out", [cfg.shard, D], dt.float32, kind="ExternalOutput").ap()

    rg = [list(range(cfg.n_cores))]

    with tile.TileContext(nc) as tc, ExitStack() as ctx:
        const = ctx.enter_context(tc.tile_pool(name="const", bufs=1))
        dram = ctx.enter_context(tc.tile_pool(name="dram", bufs=1, space="DRAM"))
        mpool = ctx.enter_context(tc.tile_pool(name="mpool", bufs=2))
        ppool = ctx.enter_context(tc.tile_pool(name="ppool", bufs=2))
        meta = ctx.enter_context(tc.tile_pool(name="meta", bufs=2))
        work = ctx.enter_context(tc.tile_pool(name="work", bufs=3))
        psum = ctx.enter_context(tc.tile_pool(name="psum", bufs=2, space="PSUM"))

        w1_sb = const.tile([D, D], dt.bfloat16)
        nc.sync.dma_start(w1_sb[:], w1[:])
        w2_sb = const.tile([D, D], dt.bfloat16)
        nc.sync.dma_start(w2_sb[:], w2[:])
        iota_sb = const.tile([P, P], dt.bfloat16)
        nc.sync.dma_start(iota_sb[:], iota[:])
        ident_sb = const.tile([P, P], dt.float32)
        nc.sync.dma_start(ident_sb[:], ident[:])
        sc1_sb = const.tile([P, cfg.tiles], dt.float32)
        nc.sync.dma_start(sc1_sb[:], sc1[:])
        sc2_sb = const.tile([P, cfg.tiles], dt.float32)
        nc.sync.dma_start(sc2_sb[:], sc2[:])

        u2_sh = dram.tile([cfg.shard, D], dt.bfloat16)
        u2_full = dram.tile([cfg.np_, D], dt.bfloat16)

        # ---------------- layer 1: streamed messages, S^T scatter ----------
        for bi in range(cfg.nb):
            mb = mpool.tile([P, cfg.kb1, D], dt.bfloat16, tag="mb")
            pb = ppool.tile([P, cfg.kb1, D], dt.bfloat16, tag="pb")
            db = meta.tile([P, cfg.kb1], dt.bfloat16, tag="db")
            nc.sync.dma_start(
                mb[:],
                m1[bi * P : (bi + 1) * P, :].rearrange("p (k d) -> p k d", d=D),
            )
            nc.sync.dma_start(db[:], dloc1[:, bi * cfg.kb1 : (bi + 1) * cfg.kb1])
            nc.vector.tensor_tensor(
                out=pb[:, :, :],
                in0=db[:, :, None].to_broadcast([P, cfg.kb1, P]),
                in1=iota_sb[:, None, :].to_broadcast([P, cfg.kb1, P]),
                op=mybir.AluOpType.is_equal,
            )
            for tb in range(cfg.b):
                t = bi * cfg.b + tb
                # S^T accumulation: ps [f, d]
                ps = psum.tile([P, D], dt.float32, tag="psS")
                for i in range(cfg.ch1):
                    k = tb * cfg.ch1 + i
                    nc.tensor.matmul(
                        ps[:],
                        lhsT=mb[:, k, :],
                        rhs=pb[:, k, :],
                        start=(i == 0),
                        stop=(i == cfg.ch1 - 1),
                    )
                sT = work.tile([P, D], dt.bfloat16, tag="sT")
                nc.vector.tensor_copy(sT[:], ps[:])
                psA = psum.tile([P, D], dt.float32, tag="psA")
                nc.tensor.matmul(
                    psA[:], lhsT=sT[:], rhs=w1_sb[:], start=True, stop=True
                )
                if with_bias:
                    bp = work.tile([P, D], dt.float32, tag="bp")
                    nc.sync.dma_start(bp[:], bpre1[t * P : (t + 1) * P, :])
                    sb = work.tile([P, D], dt.float32, tag="sb")
                    nc.vector.tensor_add(sb[:], psA[:], bp[:])
                    acc = sb
                else:
                    acc = psA
                t2 = work.tile([P, D], dt.float32, tag="t2")
                nc.scalar.activation(t2[:], acc[:], Relu, scale=sc1_sb[:, t : t + 1])
                psT = psum.tile([P, D], dt.float32, tag="psT")
                nc.tensor.transpose(psT[:], t2[:], ident_sb[:])
                tT = work.tile([P, D], dt.bfloat16, tag="tT")
                nc.vector.tensor_copy(tT[:], psT[:])
                psU = psum.tile([P, D], dt.float32, tag="psU")
                nc.tensor.matmul(
                    psU[:], lhsT=tT[:], rhs=w2_sb[:], start=True, stop=True
                )
                u2t = work.tile([P, D], dt.bfloat16, tag="u2t")
                nc.scalar.copy(u2t[:], psU[:])
                nc.sync.dma_start(u2_sh[t * P : (t + 1) * P, :], u2t[:])

        nc.gpsimd.collective_compute(
            "AllGather",
            mybir.AluOpType.bypass,
            replica_groups=rg,
            ins=[u2_sh.opt()],
            outs=[u2_full.opt()],
        )

        # ---------------- layer 2: dma_gather + S scatter -------------------
        for bi in range(cfg.nb):
            mb = mpool.tile([P, cfg.kb, D], dt.bfloat16, tag="mb")
            pb = ppool.tile([P, cfg.kb, D], dt.bfloat16, tag="pb")
            ib = meta.tile([P, cfg.grp * cfg.wcols], dt.int16, tag="ib")
            db = meta.tile([P, cfg.kb], dt.bfloat16, tag="db")
            nc.sync.dma_start(
                ib[:],
                idxs[:, bi * cfg.grp * cfg.wcols : (bi + 1) * cfg.grp * cfg.wcols],
            )
            nc.sync.dma_start(db[:], dloc[:, bi * cfg.kb : (bi + 1) * cfg.kb])
            for g in range(cfg.grp):
                nc.gpsimd.dma_gather(
                    mb[:, g * cfg.chb : (g + 1) * cfg.chb, :],
                    u2_full[g * cfg.blk : (g + 1) * cfg.blk, :],
                    ib[:, g * cfg.wcols : (g + 1) * cfg.wcols],
                    cfg.ncall,
                    cfg.ncall,
                    D,
                    single_packet=(cfg.ncall * 2 < 4096),
                    queue_num=g % 4,
                )
            nc.vector.tensor_tensor(
                out=pb[:, :, :],
                in0=db[:, :, None].to_broadcast([P, cfg.kb, P]),
                in1=iota_sb[:, None, :].to_broadcast([P, cfg.kb, P]),
                op=mybir.AluOpType.is_equal,
            )
            for tb in range(cfg.b):
                t = bi * cfg.b + tb
                ps = psum.tile([P, D], dt.float32, tag="psS")
                chunks = [
                    g * cfg.chb + tb * cfg.ch + c
                    for g in range(cfg.grp)
                    for c in range(cfg.ch)
                ]
                for i, k in enumerate(chunks):
                    nc.tensor.matmul(
                        ps[:],
                        lhsT=pb[:, k, :],
                        rhs=mb[:, k, :],
                        start=(i == 0),
                        stop=(i == len(chunks) - 1),
                    )
                acc = ps
                if with_bias:
                    bp = work.tile([P, D], dt.float32, tag="bp")
                    nc.sync.dma_start(bp[:], bpre2[t * P : (t + 1) * P, :])
                    sb = work.tile([P, D], dt.float32, tag="sb")
                    nc.vector.tensor_add(sb[:], ps[:], bp[:])
                    acc = sb
                o = work.tile([P, D], dt.float32, tag="o")
                nc.scalar.activation(o[:], acc[:], Relu, scale=sc2_sb[:, t : t + 1])
                nc.sync.dma_start(out[t * P : (t + 1) * P, :], o[:])

    nc.compile()
    return nc


def run(x, edge_index, W1, b1, W2, b2, cfg: Cfg, trace: bool = False):
    if trace:
        install_ntff_hook()
    t0 = time.time()
    in_maps, with_bias, cfg, orig_of = preprocess(x, edge_index, W1, b1, W2, b2, cfg)
    t1 = time.time()
    nc = build_program(cfg, with_bias)
    t2 = time.time()
    res = run_bass_kernel_spmd(
        nc, in_maps, core_ids=list(range(cfg.n_cores)), trace=trace
    )
    t3 = time.time()
    outs = [res.results[c]["out"] for c in range(cfg.n_cores)]
    full_new = np.concatenate(outs, axis=0)
    # un-permute: output row for original node i sits at new slot new_of[i]
    full = np.zeros((cfg.n_real, cfg.d), np.float32)
    valid = orig_of >= 0
    full[orig_of[valid]] = full_new[valid]
    LAST_INFO.clear()
    LAST_INFO.update(
        dict(
            exec_time_ns=res.exec_time_ns,
            preprocess_s=t1 - t0,
            build_compile_s=t2 - t1,
            run_s=t3 - t2,
            cfg=cfg,
            results=res,
        )
    )
    return full


def kernel(x, edge_index, W1, b1, W2, b2):
    return run(
        np.asarray(x, np.float32),
        np.asarray(edge_index),
        np.asarray(W1, np.float32),
        np.asarray(b1, np.float32),
        np.asarray(W2, np.float32),
        np.asarray(b2, np.float32),
        FULL_CFG,
    )



# revision 6
# speedup vs baseline: 1.0245x; 1.0245x over previous
"""2-layer GCN encoder (PyG GCNConv semantics) on 8 Trainium2 NeuronCores.

  out_l = relu(dinv * (A_hat @ u_l) + b_l),  u_l = (dinv * in_l) @ W_l
  A_hat includes self loops; dinv = deg^-1/2 (deg incl. self loop).

Layout: nodes are relabelled by a degree-balancing permutation, padded to
NP = 8*SHARD, and partitioned into 784 dst tiles of 128 (98 tiles per core).

Layer 1: the host pre-gathers x*dinv rows into edge order (grouped by dst
tile, padded per tile); the device streams them contiguously, scatter-adds
S^T per tile via one-hot matmuls (lhsT=M, rhs=P), then applies W1, the
relu/dinv epilogue, and W2 to produce the layer-2 source u2 directly.

Layer 2: u2 shards are AllGathered (bf16); each core then row-gathers
u2_full[src] for its dst tiles with per-(tile,group) single-packet
dma_gather calls (self-loop messages are excluded from the gather — the
self contribution is added as an identity matmul from the local u2 tile;
per-segment padding uses trailing negative indices, which the DMA skips),
scatter-adds via one-hot matmuls (lhsT=P, rhs=M), and writes the
relu(dinv*S) output tiles, one batched DMA per 7-tile batch.
"""

import time
from contextlib import ExitStack
from dataclasses import dataclass, replace

import numpy as np
import ml_dtypes

import concourse.bass as bass
import concourse.bacc as bacc
import concourse.mybir as mybir
import concourse.tile as tile
from concourse.bass_utils import run_bass_kernel_spmd

BF16 = ml_dtypes.bfloat16
FP8 = ml_dtypes.float8_e4m3
P = 128


@dataclass(frozen=True)
class Cfg:
    n_cores: int = 8
    d: int = 128
    n_real: int = 100000
    shard: int = 12544       # nodes per core, multiple of 128
    b: int = 7               # dst tiles per batch
    grp: int = 4             # layer-2 source blocks (int16 index range)
    cap: int = 640           # layer-2 edge slots per (tile, group), mult of 128
    cap1: int = 2304         # layer-1 edge slots per tile, multiple of 128
    pb_fp8: bool = False     # one-hot matrices in fp8 (exact for 0/1)

    @property
    def np_(self):
        return self.n_cores * self.shard

    @property
    def tiles(self):
        return self.shard // P

    @property
    def nb(self):
        return self.tiles // self.b

    @property
    def blk(self):
        return self.np_ // self.grp

    @property
    def ch(self):
        return self.cap // P

    @property
    def chb(self):
        return self.b * self.ch

    @property
    def kb(self):
        return self.grp * self.chb       # L2 chunks per batch

    @property
    def wcols(self):
        return self.cap // 16            # idx columns per gather call

    @property
    def ch1(self):
        return self.cap1 // P

    @property
    def kb1(self):
        return self.b * self.ch1         # L1 chunks per batch


FULL_CFG = Cfg()

LAST_INFO: dict = {}


def install_ntff_hook():
    """Provide antenv.axon_hooks (absent on this image) so that
    run_bass_kernel_spmd(trace=True) can capture NTFF profiles."""
    import sys
    import types

    if "antenv.axon_hooks" in sys.modules:
        return
    mod = types.ModuleType("antenv.axon_hooks")
    holder = [None]
    mod.set_axon_ntff_profile_hook = lambda h: holder.__setitem__(0, h)
    mod.get_axon_ntff_profile_hook = lambda: holder[0]
    sys.modules["antenv.axon_hooks"] = mod
    try:
        import antenv

        antenv.axon_hooks = mod
    except ImportError:
        pass
    try:
        from trn_agent_boot.trn_boot import _ntff_profile_via_ctypes

        hook = _ntff_profile_via_ctypes("/opt/axon/libaxon_pjrt.so")
        if hook is not None:
            mod.set_axon_ntff_profile_hook(hook)
    except Exception as e:  # profiling optional
        print(f"NTFF hook install failed: {e}")


def _check_cfg(cfg: Cfg):
    assert cfg.shard % P == 0 and cfg.tiles % cfg.b == 0
    assert cfg.cap % P == 0 and cfg.cap1 % P == 0
    assert cfg.np_ % cfg.grp == 0
    assert cfg.blk <= 32768, "relative gather indices must fit int16"
    assert cfg.cap * 2 < 4096, "single-packet gather needs idx list < 4KB"


def preprocess(x, edge_index, W1, b1, W2, b2, cfg: Cfg):
    N, D = cfg.n_real, cfg.d
    NP = cfg.np_
    assert x.shape == (N, D)

    e_src = np.asarray(edge_index[0]).astype(np.int64)
    e_dst = np.asarray(edge_index[1]).astype(np.int64)
    loops = np.arange(N, dtype=np.int64)
    src0 = np.concatenate([e_src, loops])
    dst0 = np.concatenate([e_dst, loops])

    deg0 = np.bincount(dst0, minlength=N).astype(np.float32)

    # degree-balancing relabelling: snake-deal nodes (sorted by in-degree)
    # across the tiles so every tile gets ~equal total degree.
    ntiles = NP // P
    order_by_deg = np.argsort(-deg0, kind="stable")          # real nodes
    dealt = np.full(P * ntiles, -1, np.int64)
    dealt[:N] = order_by_deg
    dealt = dealt.reshape(P, ntiles)
    dealt[1::2] = dealt[1::2, ::-1]                          # snake rounds
    # node dealt[r, t] -> new id t*128 + r
    new_of = np.full(N, -1, np.int64)
    rr, tt = np.nonzero(dealt >= 0)
    new_ids = tt * P + rr
    new_of[dealt[rr, tt]] = new_ids
    orig_of = np.full(NP, -1, np.int64)
    orig_of[new_ids] = dealt[rr, tt]

    src = new_of[src0]
    dst = new_of[dst0]

    deg = np.zeros(NP, np.float32)
    deg[new_ids] = deg0[dealt[rr, tt]]
    dinv = np.zeros(NP, np.float32)
    nz = deg > 0
    dinv[nz] = 1.0 / np.sqrt(deg[nz])

    # scaled features in new numbering (pad rows zero)
    xs = np.zeros((NP, D), np.float32)
    dinv0 = np.zeros(N, np.float32)
    dinv0[deg0 > 0] = 1.0 / np.sqrt(deg0[deg0 > 0])
    xs[new_of] = np.asarray(x, np.float32) * dinv0[:, None]
    xs_bf = xs.astype(BF16)

    nc_, nb, b, grp = cfg.n_cores, cfg.nb, cfg.b, cfg.grp

    # ---- layer 1: host-pregathered message stream, grouped by dst tile ----
    # (self-loop messages included: they ride the stream for free)
    t_of = dst >> 7
    order1 = np.argsort(t_of, kind="stable")
    d1 = dst[order1]
    s1 = src[order1]
    k1 = t_of[order1]
    cnt1 = np.bincount(k1, minlength=ntiles)
    need1 = int(np.ceil(cnt1.max() / P)) * P
    if need1 > cfg.cap1:
        cfg = replace(cfg, cap1=need1)
    starts1 = np.zeros(ntiles + 1, np.int64)
    starts1[1:] = np.cumsum(cnt1)
    pos1 = np.arange(len(s1)) - starts1[k1]
    dest1 = k1 * cfg.cap1 + pos1

    msg = np.zeros((ntiles * cfg.cap1, D), BF16)
    msg[dest1] = xs_bf[s1]
    dloc1_flat = np.full(ntiles * cfg.cap1, -1.0, BF16)
    dloc1_flat[dest1] = (d1 & 127).astype(BF16)

    ch1 = cfg.cap1 // P
    kb1 = b * ch1
    # [c, nb, tb, ch1, p, f] -> [c, nb, p, tb, ch1, f]
    m1_in = np.ascontiguousarray(
        msg.reshape(nc_, nb, b, ch1, P, D).transpose(0, 1, 4, 2, 3, 5)
    ).reshape(nc_, nb * P, kb1 * D)
    dloc1_in = np.ascontiguousarray(
        dloc1_flat.reshape(nc_, nb, b, ch1, P)
        .transpose(0, 4, 1, 2, 3)
        .reshape(nc_, P, nb * kb1)
    )

    # ---- layer 2: gather structures (no self-loops; new ids) ----
    src2 = new_of[e_src]
    dst2 = new_of[e_dst]
    key = (dst2 >> 7) * grp + src2 // cfg.blk
    order = np.argsort(key, kind="stable")
    ks = key[order]
    ss = src2[order]
    ds = dst2[order]
    nkeys = ntiles * grp
    counts = np.bincount(ks, minlength=nkeys).astype(np.int64)
    need = int(np.ceil(counts.max() / P)) * P
    if need > cfg.cap:
        cfg = replace(cfg, cap=need)
    _check_cfg(cfg)
    cap = cfg.cap
    ch = cfg.ch

    starts = np.zeros(nkeys + 1, np.int64)
    starts[1:] = np.cumsum(counts)
    pos = np.arange(len(ss)) - starts[ks]
    dest = ks * cap + pos

    epad = nkeys * cap
    # pad slots gather row 0 of the block; one-hot columns mask them out
    idx_flat = np.zeros(epad, np.int16)
    idx_flat[dest] = (ss - (ks % grp) * cfg.blk).astype(np.int16)
    dloc_flat = np.full(epad, -1.0, dtype=BF16)
    dloc_flat[dest] = (ds & 127).astype(BF16)

    idx6 = idx_flat.reshape(nc_, nb, b, grp, cfg.wcols, 16)
    wr = idx6.transpose(0, 1, 3, 2, 5, 4)          # (c, nb, grp, b, 16, wcols)
    wr = np.tile(wr, (1, 1, 1, 1, 8, 1))
    idxs_in = np.ascontiguousarray(
        wr.transpose(0, 4, 1, 2, 3, 5).reshape(nc_, P, nb * grp * b * cfg.wcols)
    )

    dl6 = dloc_flat.reshape(nc_, nb, b, grp, ch, P)
    dl = dl6.transpose(0, 1, 5, 3, 2, 4).reshape(nc_, nb, P, grp * b * ch)
    dloc_in = np.ascontiguousarray(
        dl.transpose(0, 2, 1, 3).reshape(nc_, P, nb * cfg.kb)
    )

    dinv2 = dinv * dinv
    sc1_in = np.ascontiguousarray(
        dinv2.reshape(nc_, cfg.tiles, P).transpose(0, 2, 1)
    ).astype(np.float32)
    sc2_in = np.ascontiguousarray(
        dinv.reshape(nc_, cfg.tiles, P).transpose(0, 2, 1)
    ).astype(np.float32)

    iota_in = np.tile(np.arange(P, dtype=BF16)[None, :], (P, 1))
    ident_in = np.eye(P, dtype=np.float32)
    identb_in = np.eye(P, dtype=np.float32).astype(BF16)
    w1_in = np.asarray(W1, np.float32).astype(BF16)
    w2_in = np.asarray(W2, np.float32).astype(BF16)

    b1 = np.asarray(b1, np.float32)
    b2 = np.asarray(b2, np.float32)
    with_bias = bool(np.any(b1 != 0) or np.any(b2 != 0))
    sqrtdeg = np.sqrt(deg)

    in_maps = []
    for c in range(nc_):
        m = {
            "m1": m1_in[c],
            "dloc1": dloc1_in[c],
            "w1": w1_in,
            "w2": w2_in,
            "iota": iota_in,
            "ident": ident_in,
            "identb": identb_in,
            "idxs": idxs_in[c],
            "dloc": dloc_in[c],
            "sc1": sc1_in[c],
            "sc2": sc2_in[c],
        }
        if with_bias:
            sh = slice(c * cfg.shard, (c + 1) * cfg.shard)
            m["bpre1"] = np.ascontiguousarray(np.outer(sqrtdeg[sh], b1)).astype(
                np.float32
            )
            m["bpre2"] = np.ascontiguousarray(np.outer(sqrtdeg[sh], b2)).astype(
                np.float32
            )
        in_maps.append(m)
    return in_maps, with_bias, cfg, orig_of


def build_program(cfg: Cfg, with_bias: bool):
    _check_cfg(cfg)
    D = cfg.d
    B = cfg.b * P
    dt = mybir.dt
    pb_dt = dt.float8e4 if cfg.pb_fp8 else dt.bfloat16
    Relu = mybir.ActivationFunctionType.Relu

    nc = bacc.Bacc(
        "TRN2",
        target_bir_lowering=False,
        debug=False,
        num_devices=cfg.n_cores,
        num_swdge_queues=4,
    )

    m1 = nc.dram_tensor(
        "m1", [cfg.nb * P, cfg.kb1 * D], dt.bfloat16, kind="ExternalInput"
    ).ap()
    dloc1 = nc.dram_tensor(
        "dloc1", [P, cfg.nb * cfg.kb1], dt.bfloat16, kind="ExternalInput"
    ).ap()
    w1 = nc.dram_tensor("w1", [D, D], dt.bfloat16, kind="ExternalInput").ap()
    w2 = nc.dram_tensor("w2", [D, D], dt.bfloat16, kind="ExternalInput").ap()
    iota = nc.dram_tensor("iota", [P, P], dt.bfloat16, kind="ExternalInput").ap()
    ident = nc.dram_tensor("ident", [P, P], dt.float32, kind="ExternalInput").ap()
    identb = nc.dram_tensor("identb", [P, P], dt.bfloat16, kind="ExternalInput").ap()
    idxs = nc.dram_tensor(
        "idxs", [P, cfg.nb * cfg.grp * cfg.b * cfg.wcols], dt.int16,
        kind="ExternalInput",
    ).ap()
    dloc = nc.dram_tensor(
        "dloc", [P, cfg.nb * cfg.kb], dt.bfloat16, kind="ExternalInput"
    ).ap()
    sc1 = nc.dram_tensor("sc1", [P, cfg.tiles], dt.float32, kind="ExternalInput").ap()
    sc2 = nc.dram_tensor("sc2", [P, cfg.tiles], dt.float32, kind="ExternalInput").ap()
    if with_bias:
        bpre1 = nc.dram_tensor(
            "bpre1", [cfg.shard, D], dt.float32, kind="ExternalInput"
        ).ap()
        bpre2 = nc.dram_tensor(
            "bpre2", [cfg.shard, D], dt.float32, kind="ExternalInput"
        ).ap()
    out = nc.dram_tensor("out", [cfg.shard, D], dt.float32, kind="ExternalOutput").ap()

    rg = [list(range(cfg.n_cores))]

    with tile.TileContext(nc) as tc, ExitStack() as ctx:
        const = ctx.enter_context(tc.tile_pool(name="const", bufs=1))
        dram = ctx.enter_context(tc.tile_pool(name="dram", bufs=1, space="DRAM"))
        mpool = ctx.enter_context(tc.tile_pool(name="mpool", bufs=2))
        ppool = ctx.enter_context(tc.tile_pool(name="ppool", bufs=2))
        meta = ctx.enter_context(tc.tile_pool(name="meta", bufs=3))
        spool = ctx.enter_context(tc.tile_pool(name="spool", bufs=2))
        work = ctx.enter_context(tc.tile_pool(name="work", bufs=3))
        psum = ctx.enter_context(tc.tile_pool(name="psum", bufs=2, space="PSUM"))

        w1_sb = const.tile([D, D], dt.bfloat16)
        nc.sync.dma_start(w1_sb[:], w1[:])
        w2_sb = const.tile([D, D], dt.bfloat16)
        nc.sync.dma_start(w2_sb[:], w2[:])
        iota_sb = const.tile([P, P], dt.bfloat16)
        nc.sync.dma_start(iota_sb[:], iota[:])
        ident_sb = const.tile([P, P], dt.float32)
        nc.sync.dma_start(ident_sb[:], ident[:])
        identb_sb = const.tile([P, P], dt.bfloat16)
        nc.sync.dma_start(identb_sb[:], identb[:])
        sc1_sb = const.tile([P, cfg.tiles], dt.float32)
        nc.sync.dma_start(sc1_sb[:], sc1[:])
        sc2_sb = const.tile([P, cfg.tiles], dt.float32)
        nc.sync.dma_start(sc2_sb[:], sc2[:])

        u2_sh = dram.tile([cfg.shard, D], dt.bfloat16)
        u2_full = dram.tile([cfg.np_, D], dt.bfloat16)

        # ---------------- layer 1: streamed messages, S^T scatter ----------
        for bi in range(cfg.nb):
            mb = mpool.tile([P, cfg.kb1, D], dt.bfloat16, tag="mb")
            pb = ppool.tile([P, cfg.kb1, P], pb_dt, tag="pb")
            db = meta.tile([P, cfg.kb1], dt.bfloat16, tag="db")
            u2all = spool.tile([P, cfg.b, D], dt.bfloat16, tag="u2all")
            nc.sync.dma_start(
                mb[:],
                m1[bi * P : (bi + 1) * P, :].rearrange("p (k d) -> p k d", d=D),
            )
            nc.sync.dma_start(db[:], dloc1[:, bi * cfg.kb1 : (bi + 1) * cfg.kb1])
            eng = nc.vector
            eng.tensor_tensor(
                out=pb[:, :, :],
                in0=db[:, :, None].to_broadcast([P, cfg.kb1, P]),
                in1=iota_sb[:, None, :].to_broadcast([P, cfg.kb1, P]),
                op=mybir.AluOpType.is_equal,
            )
            for tb in range(cfg.b):
                t = bi * cfg.b + tb
                # S^T accumulation: ps [f, d]
                ps = psum.tile([P, D], dt.float32, tag="psS")
                for i in range(cfg.ch1):
                    k = tb * cfg.ch1 + i
                    nc.tensor.matmul(
                        ps[:],
                        lhsT=mb[:, k, :],
                        rhs=pb[:, k, :],
                        start=(i == 0),
                        stop=(i == cfg.ch1 - 1),
                    )
                sT = work.tile([P, D], dt.bfloat16, tag="sT")
                nc.vector.tensor_copy(sT[:], ps[:])
                psA = psum.tile([P, D], dt.float32, tag="psA")
                nc.tensor.matmul(
                    psA[:], lhsT=sT[:], rhs=w1_sb[:], start=True, stop=True
                )
                if with_bias:
                    bp = work.tile([P, D], dt.float32, tag="bp")
                    nc.sync.dma_start(bp[:], bpre1[t * P : (t + 1) * P, :])
                    sb = work.tile([P, D], dt.float32, tag="sb")
                    nc.vector.tensor_add(sb[:], psA[:], bp[:])
                    acc = sb
                else:
                    acc = psA
                t2 = work.tile([P, D], dt.float32, tag="t2")
                nc.scalar.activation(t2[:], acc[:], Relu, scale=sc1_sb[:, t : t + 1])
                psT = psum.tile([P, D], dt.float32, tag="psT")
                nc.tensor.transpose(psT[:], t2[:], ident_sb[:])
                tT = work.tile([P, D], dt.bfloat16, tag="tT")
                nc.vector.tensor_copy(tT[:], psT[:])
                psU = psum.tile([P, D], dt.float32, tag="psU")
                nc.tensor.matmul(
                    psU[:], lhsT=tT[:], rhs=w2_sb[:], start=True, stop=True
                )
                nc.scalar.copy(u2all[:, tb, :], psU[:])
            nc.sync.dma_start(
                u2_sh[bi * B : (bi + 1) * B, :].rearrange("(t p) d -> p t d", p=P),
                u2all[:],
            )

        nc.gpsimd.collective_compute(
            "AllGather",
            mybir.AluOpType.bypass,
            replica_groups=rg,
            ins=[u2_sh.opt()],
            outs=[u2_full.opt()],
        )

        # ---------------- layer 2: dma_gather + S scatter -------------------
        for bi in range(cfg.nb):
            mb = mpool.tile([P, cfg.kb, D], dt.bfloat16, tag="mb")
            pb = ppool.tile([P, cfg.kb, P], pb_dt, tag="pb")
            ib = meta.tile([P, cfg.grp * cfg.b * cfg.wcols], dt.int16, tag="ib")
            db = meta.tile([P, cfg.kb], dt.bfloat16, tag="db")
            u2b = spool.tile([P, cfg.b, D], dt.bfloat16, tag="u2b")
            oall = spool.tile([P, cfg.b, D], dt.float32, tag="oall")
            nc.sync.dma_start(
                ib[:],
                idxs[
                    :,
                    bi * cfg.grp * cfg.b * cfg.wcols : (bi + 1)
                    * cfg.grp
                    * cfg.b
                    * cfg.wcols,
                ],
            )
            nc.sync.dma_start(db[:], dloc[:, bi * cfg.kb : (bi + 1) * cfg.kb])
            nc.sync.dma_start(
                u2b[:],
                u2_sh[bi * B : (bi + 1) * B, :].rearrange("(t p) d -> p t d", p=P),
            )
            for g in range(cfg.grp):
                for tb in range(cfg.b):
                    nc.gpsimd.dma_gather(
                        mb[:, g * cfg.chb + tb * cfg.ch : g * cfg.chb + (tb + 1) * cfg.ch, :],
                        u2_full[g * cfg.blk : (g + 1) * cfg.blk, :],
                        ib[:, (g * cfg.b + tb) * cfg.wcols : (g * cfg.b + tb + 1) * cfg.wcols],
                        cfg.cap,
                        cfg.cap,
                        D,
                        single_packet=True,
                        queue_num=g,
                    )
            nc.vector.tensor_tensor(
                out=pb[:, :, :],
                in0=db[:, :, None].to_broadcast([P, cfg.kb, P]),
                in1=iota_sb[:, None, :].to_broadcast([P, cfg.kb, P]),
                op=mybir.AluOpType.is_equal,
            )
            for tb in range(cfg.b):
                t = bi * cfg.b + tb
                ps = psum.tile([P, D], dt.float32, tag="psS")
                # self-loop contribution: ps[d, f] = u2[tile t][d, f]
                nc.tensor.matmul(
                    ps[:],
                    lhsT=identb_sb[:],
                    rhs=u2b[:, tb, :],
                    start=True,
                    stop=False,
                )
                chunks = [
                    g * cfg.chb + tb * cfg.ch + c
                    for g in range(cfg.grp)
                    for c in range(cfg.ch)
                ]
                for i, k in enumerate(chunks):
                    nc.tensor.matmul(
                        ps[:],
                        lhsT=pb[:, k, :],
                        rhs=mb[:, k, :],
                        start=False,
                        stop=(i == len(chunks) - 1),
                    )
                acc = ps
                if with_bias:
                    bp = work.tile([P, D], dt.float32, tag="bp")
                    nc.sync.dma_start(bp[:], bpre2[t * P : (t + 1) * P, :])
                    sb = work.tile([P, D], dt.float32, tag="sb")
                    nc.vector.tensor_add(sb[:], ps[:], bp[:])
                    acc = sb
                nc.scalar.activation(
                    oall[:, tb, :], acc[:], Relu, scale=sc2_sb[:, t : t + 1]
                )
            nc.sync.dma_start(
                out[bi * B : (bi + 1) * B, :].rearrange("(t p) d -> p t d", p=P),
                oall[:],
            )

    nc.compile()
    return nc


def run(x, edge_index, W1, b1, W2, b2, cfg: Cfg, trace: bool = False):
    if trace:
        install_ntff_hook()
    t0 = time.time()
    in_maps, with_bias, cfg, orig_of = preprocess(x, edge_index, W1, b1, W2, b2, cfg)
    t1 = time.time()
    nc = build_program(cfg, with_bias)
    t2 = time.time()
    res = run_bass_kernel_spmd(
        nc, in_maps, core_ids=list(range(cfg.n_cores)), trace=trace
    )
    t3 = time.time()
    outs = [res.results[c]["out"] for c in range(cfg.n_cores)]
    full_new = np.concatenate(outs, axis=0)
    # un-permute: output row for original node i sits at new slot new_of[i]
    full = np.zeros((cfg.n_real, cfg.d), np.float32)
    valid = orig_of >= 0
    full[orig_of[valid]] = full_new[valid]
    LAST_INFO.clear()
    LAST_INFO.update(
        dict(
            exec_time_ns=res.exec_time_ns,
            preprocess_s=t1 - t0,
            build_compile_s=t2 - t1,
            run_s=t3 - t2,
            cfg=cfg,
            results=res,
        )
    )
    return full


def kernel(x, edge_index, W1, b1, W2, b2):
    return run(
        np.asarray(x, np.float32),
        np.asarray(edge_index),
        np.asarray(W1, np.float32),
        np.asarray(b1, np.float32),
        np.asarray(W2, np.float32),
        np.asarray(b2, np.float32),
        FULL_CFG,
    )


# revision 7
# speedup vs baseline: 1.3516x; 1.3192x over previous
"""2-layer GCN encoder (PyG GCNConv semantics) on 8 Trainium2 NeuronCores.

  out_l = relu(dinv * (A_hat @ u_l) + b_l),  u_l = (dinv * in_l) @ W_l
  A_hat includes self loops; dinv = deg^-1/2 (deg incl. self loop).

Layout: nodes are relabelled by a degree-balancing permutation, padded to
NP = 8*SHARD, and partitioned into 784 dst tiles of 128 (98 tiles per core).

Layer 1: the host pre-gathers x*dinv rows into edge order (grouped by dst
tile, padded per tile); the device streams them contiguously, scatter-adds
S^T per tile via one-hot matmuls (lhsT=M, rhs=P), then applies W1, the
relu/dinv epilogue, and W2 to produce the layer-2 source u2 directly.

Layer 2: u2 shards are AllGathered (bf16); each core then row-gathers
u2_full[src] for its dst tiles with per-(tile,group) single-packet
dma_gather calls (self-loop messages are excluded from the gather — the
self contribution is added as an identity matmul from the local u2 tile;
per-segment padding uses trailing negative indices, which the DMA skips),
scatter-adds via one-hot matmuls (lhsT=P, rhs=M), and writes the
relu(dinv*S) output tiles, one batched DMA per 7-tile batch.
"""

import time
from contextlib import ExitStack
from dataclasses import dataclass, replace

import numpy as np
import ml_dtypes

import concourse.bass as bass
import concourse.bacc as bacc
import concourse.mybir as mybir
import concourse.tile as tile
from concourse.bass_utils import run_bass_kernel_spmd

BF16 = ml_dtypes.bfloat16
FP8 = ml_dtypes.float8_e4m3
P = 128


@dataclass(frozen=True)
class Cfg:
    n_cores: int = 8
    d: int = 128
    n_real: int = 100000
    shard: int = 12544       # nodes per core, multiple of 128
    b: int = 7               # dst tiles per batch
    grp: int = 7             # layer-2 source blocks (int16 index range)
    cap: int = 384           # layer-2 edge slots per (tile, group), mult of 128
    cap1: int = 2304         # layer-1 edge slots per tile, multiple of 128
    pb_fp8: bool = False     # one-hot matrices in fp8 (exact for 0/1)

    @property
    def np_(self):
        return self.n_cores * self.shard

    @property
    def tiles(self):
        return self.shard // P

    @property
    def nb(self):
        return self.tiles // self.b

    @property
    def blk(self):
        return self.np_ // self.grp

    @property
    def ch(self):
        return self.cap // P

    @property
    def chb(self):
        return self.b * self.ch

    @property
    def kb(self):
        return self.grp * self.chb       # L2 chunks per batch

    @property
    def qrows(self):
        return self.shard // self.grp    # u2 rows per core per source block

    @property
    def wcols(self):
        return self.b * self.cap // 16   # idx columns per gather call

    @property
    def ch1(self):
        return self.cap1 // P

    @property
    def kb1(self):
        return self.b * self.ch1         # L1 chunks per batch


FULL_CFG = Cfg()

LAST_INFO: dict = {}


def install_ntff_hook():
    """Provide antenv.axon_hooks (absent on this image) so that
    run_bass_kernel_spmd(trace=True) can capture NTFF profiles."""
    import sys
    import types

    if "antenv.axon_hooks" in sys.modules:
        return
    mod = types.ModuleType("antenv.axon_hooks")
    holder = [None]
    mod.set_axon_ntff_profile_hook = lambda h: holder.__setitem__(0, h)
    mod.get_axon_ntff_profile_hook = lambda: holder[0]
    sys.modules["antenv.axon_hooks"] = mod
    try:
        import antenv

        antenv.axon_hooks = mod
    except ImportError:
        pass
    try:
        from trn_agent_boot.trn_boot import _ntff_profile_via_ctypes

        hook = _ntff_profile_via_ctypes("/opt/axon/libaxon_pjrt.so")
        if hook is not None:
            mod.set_axon_ntff_profile_hook(hook)
    except Exception as e:  # profiling optional
        print(f"NTFF hook install failed: {e}")


def _check_cfg(cfg: Cfg):
    assert cfg.shard % P == 0 and cfg.tiles % cfg.b == 0
    assert cfg.cap % P == 0 and cfg.cap1 % P == 0
    assert cfg.np_ % cfg.grp == 0
    assert cfg.blk <= 32768, "relative gather indices must fit int16"
    assert cfg.shard % cfg.grp == 0 and cfg.qrows % (2 * cfg.b * 128) == 0, (
        "source blocks must align to pairs of L1 batches"
    )


def preprocess(x, edge_index, W1, b1, W2, b2, cfg: Cfg):
    N, D = cfg.n_real, cfg.d
    NP = cfg.np_
    assert x.shape == (N, D)

    e_src = np.asarray(edge_index[0]).astype(np.int64)
    e_dst = np.asarray(edge_index[1]).astype(np.int64)
    loops = np.arange(N, dtype=np.int64)
    src0 = np.concatenate([e_src, loops])
    dst0 = np.concatenate([e_dst, loops])

    deg0 = np.bincount(dst0, minlength=N).astype(np.float32)

    # degree-balancing relabelling: snake-deal nodes (sorted by in-degree)
    # across the tiles so every tile gets ~equal total degree.
    ntiles = NP // P
    order_by_deg = np.argsort(-deg0, kind="stable")          # real nodes
    dealt = np.full(P * ntiles, -1, np.int64)
    dealt[:N] = order_by_deg
    dealt = dealt.reshape(P, ntiles)
    dealt[1::2] = dealt[1::2, ::-1]                          # snake rounds
    # node dealt[r, t] -> new id t*128 + r
    new_of = np.full(N, -1, np.int64)
    rr, tt = np.nonzero(dealt >= 0)
    new_ids = tt * P + rr
    new_of[dealt[rr, tt]] = new_ids
    orig_of = np.full(NP, -1, np.int64)
    orig_of[new_ids] = dealt[rr, tt]

    src = new_of[src0]
    dst = new_of[dst0]

    deg = np.zeros(NP, np.float32)
    deg[new_ids] = deg0[dealt[rr, tt]]
    dinv = np.zeros(NP, np.float32)
    nz = deg > 0
    dinv[nz] = 1.0 / np.sqrt(deg[nz])

    # scaled features in new numbering (pad rows zero)
    xs = np.zeros((NP, D), np.float32)
    dinv0 = np.zeros(N, np.float32)
    dinv0[deg0 > 0] = 1.0 / np.sqrt(deg0[deg0 > 0])
    xs[new_of] = np.asarray(x, np.float32) * dinv0[:, None]
    xs_bf = xs.astype(BF16)

    nc_, nb, b, grp = cfg.n_cores, cfg.nb, cfg.b, cfg.grp

    # ---- layer 1: host-pregathered message stream, grouped by dst tile ----
    # (self-loop messages included: they ride the stream for free)
    t_of = dst >> 7
    order1 = np.argsort(t_of, kind="stable")
    d1 = dst[order1]
    s1 = src[order1]
    k1 = t_of[order1]
    cnt1 = np.bincount(k1, minlength=ntiles)
    need1 = int(np.ceil(cnt1.max() / P)) * P
    if need1 > cfg.cap1:
        cfg = replace(cfg, cap1=need1)
    starts1 = np.zeros(ntiles + 1, np.int64)
    starts1[1:] = np.cumsum(cnt1)
    pos1 = np.arange(len(s1)) - starts1[k1]
    dest1 = k1 * cfg.cap1 + pos1

    msg = np.zeros((ntiles * cfg.cap1, D), BF16)
    msg[dest1] = xs_bf[s1]
    dloc1_flat = np.full(ntiles * cfg.cap1, -1.0, BF16)
    dloc1_flat[dest1] = (d1 & 127).astype(BF16)

    ch1 = cfg.cap1 // P
    kb1 = b * ch1
    # [c, nb, tb, ch1, p, f] -> [c, nb, p, tb, ch1, f]
    m1_in = np.ascontiguousarray(
        msg.reshape(nc_, nb, b, ch1, P, D).transpose(0, 1, 4, 2, 3, 5)
    ).reshape(nc_, nb * P, kb1 * D)
    dloc1_in = np.ascontiguousarray(
        dloc1_flat.reshape(nc_, nb, b, ch1, P)
        .transpose(0, 4, 1, 2, 3)
        .reshape(nc_, P, nb * kb1)
    )

    # ---- layer 2: gather structures (no self-loops; new ids) ----
    src2 = new_of[e_src]
    dst2 = new_of[e_dst]
    s_core = src2 // cfg.shard
    s_loc = src2 % cfg.shard
    s_q = s_loc // cfg.qrows
    key = (dst2 >> 7) * grp + s_q
    order = np.argsort(key, kind="stable")
    ks = key[order]
    ss = src2[order]
    ds = dst2[order]
    nkeys = ntiles * grp
    counts = np.bincount(ks, minlength=nkeys).astype(np.int64)
    need = int(np.ceil(counts.max() / P)) * P
    if need > cfg.cap:
        cfg = replace(cfg, cap=need)
    _check_cfg(cfg)
    cap = cfg.cap
    ch = cfg.ch

    starts = np.zeros(nkeys + 1, np.int64)
    starts[1:] = np.cumsum(counts)
    pos = np.arange(len(ss)) - starts[ks]
    dest = ks * cap + pos

    epad = nkeys * cap
    # pad slots gather row 0 of the block; one-hot columns mask them out
    idx_flat = np.zeros(epad, np.int16)
    sq = ks % grp
    rel = (ss // cfg.shard) * cfg.qrows + (ss % cfg.shard) - sq * cfg.qrows
    idx_flat[dest] = rel.astype(np.int16)
    dloc_flat = np.full(epad, -1.0, dtype=BF16)
    dloc_flat[dest] = (ds & 127).astype(BF16)

    idx5 = idx_flat.reshape(nc_, nb, b, grp, cap)
    idx_call = idx5.transpose(0, 1, 3, 2, 4).reshape(nc_, nb, grp, b * cap)
    wr = idx_call.reshape(nc_, nb, grp, cfg.wcols, 16).transpose(0, 1, 2, 4, 3)
    wr = np.tile(wr, (1, 1, 1, 8, 1))
    idxs_in = np.ascontiguousarray(
        wr.transpose(0, 3, 1, 2, 4).reshape(nc_, P, nb * grp * cfg.wcols)
    )

    dl6 = dloc_flat.reshape(nc_, nb, b, grp, ch, P)
    dl = dl6.transpose(0, 1, 5, 3, 2, 4).reshape(nc_, nb, P, grp * b * ch)
    dloc_in = np.ascontiguousarray(
        dl.transpose(0, 2, 1, 3).reshape(nc_, P, nb * cfg.kb)
    )

    dinv2 = dinv * dinv
    sc1_in = np.ascontiguousarray(
        dinv2.reshape(nc_, cfg.tiles, P).transpose(0, 2, 1)
    ).astype(np.float32)
    sc2_in = np.ascontiguousarray(
        dinv.reshape(nc_, cfg.tiles, P).transpose(0, 2, 1)
    ).astype(np.float32)

    iota_in = np.tile(np.arange(P, dtype=BF16)[None, :], (P, 1))
    ident_in = np.eye(P, dtype=np.float32)
    identb_in = np.eye(P, dtype=np.float32).astype(BF16)
    w1_in = np.asarray(W1, np.float32).astype(BF16)
    w2_in = np.asarray(W2, np.float32).astype(BF16)

    b1 = np.asarray(b1, np.float32)
    b2 = np.asarray(b2, np.float32)
    with_bias = bool(np.any(b1 != 0) or np.any(b2 != 0))
    sqrtdeg = np.sqrt(deg)

    in_maps = []
    for c in range(nc_):
        m = {
            "m1": m1_in[c],
            "dloc1": dloc1_in[c],
            "w1": w1_in,
            "w2": w2_in,
            "iota": iota_in,
            "ident": ident_in,
            "identb": identb_in,
            "idxs": idxs_in[c],
            "dloc": dloc_in[c],
            "sc1": sc1_in[c],
            "sc2": sc2_in[c],
        }
        if with_bias:
            sh = slice(c * cfg.shard, (c + 1) * cfg.shard)
            m["bpre1"] = np.ascontiguousarray(np.outer(sqrtdeg[sh], b1)).astype(
                np.float32
            )
            m["bpre2"] = np.ascontiguousarray(np.outer(sqrtdeg[sh], b2)).astype(
                np.float32
            )
        in_maps.append(m)
    return in_maps, with_bias, cfg, orig_of


def build_program(cfg: Cfg, with_bias: bool):
    _check_cfg(cfg)
    D = cfg.d
    B = cfg.b * P
    dt = mybir.dt
    pb_dt = dt.float8e4 if cfg.pb_fp8 else dt.bfloat16
    Relu = mybir.ActivationFunctionType.Relu

    nc = bacc.Bacc(
        "TRN2",
        target_bir_lowering=False,
        debug=False,
        num_devices=cfg.n_cores,
        num_swdge_queues=4,
    )

    m1 = nc.dram_tensor(
        "m1", [cfg.nb * P, cfg.kb1 * D], dt.bfloat16, kind="ExternalInput"
    ).ap()
    dloc1 = nc.dram_tensor(
        "dloc1", [P, cfg.nb * cfg.kb1], dt.bfloat16, kind="ExternalInput"
    ).ap()
    w1 = nc.dram_tensor("w1", [D, D], dt.bfloat16, kind="ExternalInput").ap()
    w2 = nc.dram_tensor("w2", [D, D], dt.bfloat16, kind="ExternalInput").ap()
    iota = nc.dram_tensor("iota", [P, P], dt.bfloat16, kind="ExternalInput").ap()
    ident = nc.dram_tensor("ident", [P, P], dt.float32, kind="ExternalInput").ap()
    identb = nc.dram_tensor("identb", [P, P], dt.bfloat16, kind="ExternalInput").ap()
    idxs = nc.dram_tensor(
        "idxs", [P, cfg.nb * cfg.grp * cfg.wcols], dt.int16,
        kind="ExternalInput",
    ).ap()
    dloc = nc.dram_tensor(
        "dloc", [P, cfg.nb * cfg.kb], dt.bfloat16, kind="ExternalInput"
    ).ap()
    sc1 = nc.dram_tensor("sc1", [P, cfg.tiles], dt.float32, kind="ExternalInput").ap()
    sc2 = nc.dram_tensor("sc2", [P, cfg.tiles], dt.float32, kind="ExternalInput").ap()
    if with_bias:
        bpre1 = nc.dram_tensor(
            "bpre1", [cfg.shard, D], dt.float32, kind="ExternalInput"
        ).ap()
        bpre2 = nc.dram_tensor(
            "bpre2", [cfg.shard, D], dt.float32, kind="ExternalInput"
        ).ap()
    out = nc.dram_tensor("out", [cfg.shard, D], dt.float32, kind="ExternalOutput").ap()

    rg = [list(range(cfg.n_cores))]

    with tile.TileContext(nc) as tc, ExitStack() as ctx:
        const = ctx.enter_context(tc.tile_pool(name="const", bufs=1))
        dram = ctx.enter_context(tc.tile_pool(name="dram", bufs=1, space="DRAM"))
        mpool = ctx.enter_context(tc.tile_pool(name="mpool", bufs=2))
        ppool = ctx.enter_context(tc.tile_pool(name="ppool", bufs=2))
        meta = ctx.enter_context(tc.tile_pool(name="meta", bufs=3))
        spool = ctx.enter_context(tc.tile_pool(name="spool", bufs=2))
        work = ctx.enter_context(tc.tile_pool(name="work", bufs=3))
        psum = ctx.enter_context(tc.tile_pool(name="psum", bufs=2, space="PSUM"))

        w1_sb = const.tile([D, D], dt.bfloat16)
        nc.sync.dma_start(w1_sb[:], w1[:])
        w2_sb = const.tile([D, D], dt.bfloat16)
        nc.sync.dma_start(w2_sb[:], w2[:])
        iota_sb = const.tile([P, P], dt.bfloat16)
        nc.sync.dma_start(iota_sb[:], iota[:])
        ident_sb = const.tile([P, P], dt.float32)
        nc.sync.dma_start(ident_sb[:], ident[:])
        identb_sb = const.tile([P, P], dt.bfloat16)
        nc.sync.dma_start(identb_sb[:], identb[:])
        sc1_sb = const.tile([P, cfg.tiles], dt.float32)
        nc.sync.dma_start(sc1_sb[:], sc1[:])
        sc2_sb = const.tile([P, cfg.tiles], dt.float32)
        nc.sync.dma_start(sc2_sb[:], sc2[:])

        u2_in = [
            dram.tile([cfg.qrows, D], dt.bfloat16, name=f"u2in{q}")
            for q in range(cfg.grp)
        ]
        u2_out = [
            dram.tile([cfg.qrows * cfg.n_cores, D], dt.bfloat16, name=f"u2out{q}")
            for q in range(cfg.grp)
        ]
        bpb = 2 * cfg.b * P  # L1 batches per source block = qrows rows

        # ---------------- layer 1: streamed messages, S^T scatter ----------
        for bi in range(cfg.nb):
            mb = mpool.tile([P, cfg.kb1, D], dt.bfloat16, tag="mb")
            pb = ppool.tile([P, cfg.kb1, P], pb_dt, tag="pb")
            db = meta.tile([P, cfg.kb1], dt.bfloat16, tag="db")
            u2all = spool.tile([P, cfg.b, D], dt.bfloat16, tag="u2all")
            nc.sync.dma_start(
                mb[:],
                m1[bi * P : (bi + 1) * P, :].rearrange("p (k d) -> p k d", d=D),
            )
            nc.sync.dma_start(db[:], dloc1[:, bi * cfg.kb1 : (bi + 1) * cfg.kb1])
            eng = nc.vector
            eng.tensor_tensor(
                out=pb[:, :, :],
                in0=db[:, :, None].to_broadcast([P, cfg.kb1, P]),
                in1=iota_sb[:, None, :].to_broadcast([P, cfg.kb1, P]),
                op=mybir.AluOpType.is_equal,
            )
            for tb in range(cfg.b):
                t = bi * cfg.b + tb
                # S^T accumulation: ps [f, d]
                ps = psum.tile([P, D], dt.float32, tag="psS")
                for i in range(cfg.ch1):
                    k = tb * cfg.ch1 + i
                    nc.tensor.matmul(
                        ps[:],
                        lhsT=mb[:, k, :],
                        rhs=pb[:, k, :],
                        start=(i == 0),
                        stop=(i == cfg.ch1 - 1),
                    )
                sT = work.tile([P, D], dt.bfloat16, tag="sT")
                nc.vector.tensor_copy(sT[:], ps[:])
                psA = psum.tile([P, D], dt.float32, tag="psA")
                nc.tensor.matmul(
                    psA[:], lhsT=sT[:], rhs=w1_sb[:], start=True, stop=True
                )
                if with_bias:
                    bp = work.tile([P, D], dt.float32, tag="bp")
                    nc.sync.dma_start(bp[:], bpre1[t * P : (t + 1) * P, :])
                    sb = work.tile([P, D], dt.float32, tag="sb")
                    nc.vector.tensor_add(sb[:], psA[:], bp[:])
                    acc = sb
                else:
                    acc = psA
                t2 = work.tile([P, D], dt.float32, tag="t2")
                nc.scalar.activation(t2[:], acc[:], Relu, scale=sc1_sb[:, t : t + 1])
                psT = psum.tile([P, D], dt.float32, tag="psT")
                nc.tensor.transpose(psT[:], t2[:], ident_sb[:])
                tT = work.tile([P, D], dt.bfloat16, tag="tT")
                nc.vector.tensor_copy(tT[:], psT[:])
                psU = psum.tile([P, D], dt.float32, tag="psU")
                nc.tensor.matmul(
                    psU[:], lhsT=tT[:], rhs=w2_sb[:], start=True, stop=True
                )
                nc.scalar.copy(u2all[:, tb, :], psU[:])
            nc.sync.dma_start(
                u2_in[bi // 2][(bi % 2) * B : (bi % 2) * B + B, :].rearrange(
                    "(t p) d -> p t d", p=P
                ),
                u2all[:],
            )

        # ---------------- layer 2: chunked AllGather + dma_gather ----------
        ag_emitted = [False] * cfg.grp
        for bi in range(cfg.nb):
            mb = mpool.tile([P, cfg.kb, D], dt.bfloat16, tag="mb")
            pb = ppool.tile([P, cfg.kb, P], pb_dt, tag="pb")
            ib = meta.tile([P, cfg.grp * cfg.wcols], dt.int16, tag="ib")
            db = meta.tile([P, cfg.kb], dt.bfloat16, tag="db")
            u2b = spool.tile([P, cfg.b, D], dt.bfloat16, tag="u2b")
            oall = spool.tile([P, cfg.b, D], dt.float32, tag="oall")
            nc.sync.dma_start(
                ib[:],
                idxs[:, bi * cfg.grp * cfg.wcols : (bi + 1) * cfg.grp * cfg.wcols],
            )
            nc.sync.dma_start(db[:], dloc[:, bi * cfg.kb : (bi + 1) * cfg.kb])
            nc.sync.dma_start(
                u2b[:],
                u2_in[bi // 2][(bi % 2) * B : (bi % 2) * B + B, :].rearrange(
                    "(t p) d -> p t d", p=P
                ),
            )
            ncall = cfg.b * cfg.cap
            for g in range(cfg.grp):
                if not ag_emitted[g]:
                    nc.gpsimd.collective_compute(
                        "AllGather",
                        mybir.AluOpType.bypass,
                        replica_groups=rg,
                        ins=[u2_in[g].opt()],
                        outs=[u2_out[g].opt()],
                    )
                    ag_emitted[g] = True
                nc.gpsimd.dma_gather(
                    mb[:, g * cfg.chb : (g + 1) * cfg.chb, :],
                    u2_out[g][:],
                    ib[:, g * cfg.wcols : (g + 1) * cfg.wcols],
                    ncall,
                    ncall,
                    D,
                    single_packet=(ncall * 2 < 4096),
                    queue_num=g % 4,
                )
            nc.vector.tensor_tensor(
                out=pb[:, :, :],
                in0=db[:, :, None].to_broadcast([P, cfg.kb, P]),
                in1=iota_sb[:, None, :].to_broadcast([P, cfg.kb, P]),
                op=mybir.AluOpType.is_equal,
            )
            for tb in range(cfg.b):
                t = bi * cfg.b + tb
                ps = psum.tile([P, D], dt.float32, tag="psS")
                # self-loop contribution: ps[d, f] = u2[tile t][d, f]
                nc.tensor.matmul(
                    ps[:],
                    lhsT=identb_sb[:],
                    rhs=u2b[:, tb, :],
                    start=True,
                    stop=False,
                )
                chunks = [
                    g * cfg.chb + tb * cfg.ch + c
                    for g in range(cfg.grp)
                    for c in range(cfg.ch)
                ]
                for i, k in enumerate(chunks):
                    nc.tensor.matmul(
                        ps[:],
                        lhsT=pb[:, k, :],
                        rhs=mb[:, k, :],
                        start=False,
                        stop=(i == len(chunks) - 1),
                    )
                acc = ps
                if with_bias:
                    bp = work.tile([P, D], dt.float32, tag="bp")
                    nc.sync.dma_start(bp[:], bpre2[t * P : (t + 1) * P, :])
                    sb = work.tile([P, D], dt.float32, tag="sb")
                    nc.vector.tensor_add(sb[:], ps[:], bp[:])
                    acc = sb
                nc.scalar.activation(
                    oall[:, tb, :], acc[:], Relu, scale=sc2_sb[:, t : t + 1]
                )
            nc.sync.dma_start(
                out[bi * B : (bi + 1) * B, :].rearrange("(t p) d -> p t d", p=P),
                oall[:],
            )

    nc.compile()
    return nc


def run(x, edge_index, W1, b1, W2, b2, cfg: Cfg, trace: bool = False):
    if trace:
        install_ntff_hook()
    t0 = time.time()
    in_maps, with_bias, cfg, orig_of = preprocess(x, edge_index, W1, b1, W2, b2, cfg)
    t1 = time.time()
    nc = build_program(cfg, with_bias)
    t2 = time.time()
    res = run_bass_kernel_spmd(
        nc, in_maps, core_ids=list(range(cfg.n_cores)), trace=trace
    )
    t3 = time.time()
    outs = [res.results[c]["out"] for c in range(cfg.n_cores)]
    full_new = np.concatenate(outs, axis=0)
    # un-permute: output row for original node i sits at new slot new_of[i]
    full = np.zeros((cfg.n_real, cfg.d), np.float32)
    valid = orig_of >= 0
    full[orig_of[valid]] = full_new[valid]
    LAST_INFO.clear()
    LAST_INFO.update(
        dict(
            exec_time_ns=res.exec_time_ns,
            preprocess_s=t1 - t0,
            build_compile_s=t2 - t1,
            run_s=t3 - t2,
            cfg=cfg,
            results=res,
        )
    )
    return full


def kernel(x, edge_index, W1, b1, W2, b2):
    return run(
        np.asarray(x, np.float32),
        np.asarray(edge_index),
        np.asarray(W1, np.float32),
        np.asarray(b1, np.float32),
        np.asarray(W2, np.float32),
        np.asarray(b2, np.float32),
        FULL_CFG,
    )
